# revision 32
# baseline (speedup 1.0000x reference)
"""Trainium2 Bass kernel for nn_ExactSpectralHead (sparse resonance attention).

Reference computation (per batch element b):
    q = x @ Wq.T; k = x @ Wk.T; v = x @ Wv.T          # [T, H]
    s = (q @ k.T) * C**-0.5 + resonance_bias          # [T, T]
    s = where(allowed, s, -inf); p = softmax(s, -1)
    out = p @ v                                        # [T, H]

Strategy (8 NeuronCores, data-parallel over batch B=8, one b per core):
  - Host folds bias+mask into EB = exp(bias) * allowed (exact: exp(log1p(r)) = 1+r),
    so p_raw = exp(s_qk * scale) * EB with no -inf handling and exact zeros.
    Scores are bounded (|s|<~5), so no max-subtraction is needed; normalization is
    deferred to after the PV matmul via a row-sum.
  - Everything is computed in a transposed layout so that every matmul contracts
    over the partition dim with zero on-device transposes:
      xT [C, T] (host-transposed), QT/KT = W.T^T @ xT -> [H, T],
      ST[tk, tq] = KT.T @ QT, PT = exp(ST*scale) * EBT,
      OT[h, tq] += V[tk,:].T @ PT[tk, tq]   (V in natural [T, H] layout),
      rowsum[tq] = ones.T @ sum_i PT_i, out = (OT / rowsum).T (transpose on host).
  - bf16 matmul inputs (1 cycle/row on the PE), fp32 PSUM accumulation.
  - Causal block skipping: tiles with tk_chunk > tq_block are never touched.
"""

import sys

sys.path.insert(0, "/opt/trn_rl_repo")

import numpy as np
import ml_dtypes

import concourse.bass as bass
import concourse.tile as tile
import concourse.mybir as mybir

# ----------------------------------------------------------------------------
# Workaround for walrus codegen "Too many sync wait commands" on the
# TileContext tail Drain: split the global-clock sem waits across multiple SP
# NOP instructions instead of attaching them all to the single Drain.
from concourse.vector_clock import ScopedClock, VectorClock


def _split_drain_and_barrier(self, tick_clock, wait_clock):
    """Cheap kernel tail: per-proc sem waits split across SP NOPs (walrus
    one-wait-per-instruction limit), then a regular-semaphore all-engine
    completion barrier (the stock EVSEM butterfly costs ~1.5-4us per hop),
    then GpSimd clears the tile semaphores. The next NEFF execution cannot
    start until every engine stream (including the clear) retires, so no
    trailing barrier is needed."""
    import concourse.mybir as _mybir

    nc = self.nc
    gc = tick_clock.global_clock
    n = len(gc)
    for p in range(n):
        t = gc[p]
        if t > 0:
            nop = nc.sync.nop(nofuse=True, hint=f"drain_wait_{p}")
            vc = VectorClock([t if i == p else 0 for i in range(n)])
            wait_clock.add_sem_waits(nop.ins, ScopedClock({None: vc}))

    tail_sem = nc.alloc_semaphore("tile_tail_sem")
    n_signals = 0
    for etype, eng in nc.engines.items():
        if etype == _mybir.EngineType.Pool:
            continue
        eng.drain(fusable=False)
        eng.sem_inc(tail_sem, 1)
        n_signals += 1
    nc.gpsimd.wait_ge(tail_sem, n_signals)
    assert self.sems is not None
    popped = nc._tile_sem_poison_stack.pop()
    assert popped is self._sem_poison
    nc.clear_and_free_semaphores(list(self.sems.allocated().values()))
    nc.gpsimd.sem_clear(range(tail_sem.num, tail_sem.num + 1))


tile.TileContext._drain_and_barrier = _split_drain_and_barrier
# ----------------------------------------------------------------------------

def _split_excess_waits(nc, max_waits=1):
    """Walrus codegen in this toolchain supports only one sem-wait per
    instruction; hoist excess waits onto preceding same-engine NOPs."""
    for f in nc.m.functions:
        for bb in f.blocks:
            new = []
            changed = False
            for inst in bb.instructions:
                if isinstance(inst, mybir.InstEventSemaphore):
                    # EventSemaphore ops measure ~3-5us on HW; their barrier
                    # semantics live entirely in sync_info (regular sems), so
                    # NoOps with the same sync_info are equivalent and fast.
                    # Waits and updates go on separate NoOps (wait first) to
                    # satisfy the no_semaphore_value_conflict ISA check.
                    si = inst.sync_info
                    changed = True
                    w = list(si.on_wait) if si else []
                    u = list(si.on_update) if si else []
                    if w:
                        new.append(
                            mybir.InstNoOp(
                                name=f"{inst.name}-wait",
                                engine=inst.engine,
                                bass_nofuse=True,
                                sync_info=mybir.SyncInfo(on_wait=w, on_update=[]),
                            )
                        )
                    new.append(
                        mybir.InstNoOp(
                            name=inst.name,
                            engine=inst.engine,
                            bass_nofuse=True,
                            sync_info=mybir.SyncInfo(on_wait=[], on_update=u),
                        )
                    )
                    continue
                si = inst.sync_info
                waits = list(si.on_wait) if si is not None else []
                if len(waits) > max_waits:
                    changed = True
                    excess, keep = waits[:-max_waits], waits[-max_waits:]
                    for k, w in enumerate(excess):
                        new.append(
                            mybir.InstNoOp(
                                name=f"{inst.name}-w{k}",
                                engine=inst.engine,
                                bass_nofuse=True,
                                sync_info=mybir.SyncInfo(on_wait=[w], on_update=[]),
                            )
                        )
                    inst.sync_info = mybir.SyncInfo(
                        on_wait=keep, on_update=list(si.on_update)
                    )
                new.append(inst)
            if changed:
                bb.instructions = new


B, T, C, H = 8, 2048, 1024, 128
NCORES = 8
SCALE = float(C) ** -0.5
P = 128
TQ = 512                 # tq block width (matmul moving dim)
NJ = T // TQ             # 4 tq blocks
NC_CHUNK = C // P        # 8 contraction chunks over channels
NK = T // P              # 16 tk chunks
BF16 = mybir.dt.bfloat16
FP8 = mybir.dt.float8e4
F32 = mybir.dt.float32
F32R = mybir.dt.float32r

_nc_cache = None


def _build_nc():
    nc = bass.Bass()
    # xT tiled: [jt, c, 128, TQ] so each (c, jt) load is one contiguous 128KB
    # per (jt, half): contiguous 512KB, partition-major [p, 4c, q]
    xTt = nc.declare_dram_parameter("xTt", [NJ, 2, P, NC_CHUNK // 2, TQ], BF16, isOutput=False)
    wqT = nc.declare_dram_parameter("wqT", [C, H], BF16, isOutput=False)
    wkT = nc.declare_dram_parameter("wkT", [C, H], BF16, isOutput=False)
    wvT = nc.declare_dram_parameter("wvT", [C, H], BF16, isOutput=False)
    # ebT quad-tiled: [j, i4, 128, 4*TQ] with ebt[j,i4,p,k*TQ+q] = EB.T[128*(4*i4+k)+p, j*TQ+q]
    ebt = nc.declare_dram_parameter("ebt", [NJ, NK // 4, P, 4 * TQ], BF16, isOutput=False)
    # output tiled: [j, 128, TQ] (host reassembles)
    outt = nc.declare_dram_parameter("outt", [NJ, H, TQ], F32, isOutput=True)

    wT3 = {
        "q": wqT.rearrange("(o p) h -> p o h", p=P),
        "k": wkT.rearrange("(o p) h -> p o h", p=P),
        "v": wvT.rearrange("(o p) h -> p o h", p=P),
    }

    with tile.TileContext(nc) as tc:
        with (
            tc.tile_pool(name="const", bufs=1) as const,
            tc.tile_pool(name="qkv_psum", bufs=2, space="PSUM") as qkv_psum,
            tc.tile_pool(name="st_psum", bufs=2, space="PSUM") as st_psum_pool,
            tc.tile_pool(name="ot_psum", bufs=1, space="PSUM") as ot_psum_pool,
            tc.tile_pool(name="rs_psum", bufs=1, space="PSUM") as rs_psum_pool,
            tc.tile_pool(name="pt", bufs=NK // 2 + 2) as pt_pool,
            tc.tile_pool(name="eb", bufs=4) as eb_pool,
            tc.tile_pool(name="outs", bufs=2) as out_pool,
        ):
            # ---------- persistent SBUF tensors ----------
            w_sb = {}
            for w in ("q", "k", "v"):
                w_sb[w] = const.tile([P, NC_CHUNK, H], BF16, tag=f"w_{w}", name=f"w_{w}_sb")
            with tc.high_priority():
                nc.sync.dma_start(w_sb["q"][:], wT3["q"][:])

            # xT loaded per (tq-block, c-chunk) in consumption order
            xT_sb = const.tile([P, NC_CHUNK, T], BF16, tag="xT", name="xT_sb")
            with tc.high_priority():
                # jt0 in quarters so the first Q matmuls start early
                for qtr in range(4):
                    eng = nc.sync if qtr % 2 == 0 else nc.scalar
                    eng.dma_start(
                        xT_sb[:, qtr * 2:(qtr + 1) * 2, 0:TQ],
                        xTt[0, qtr // 2][:, (qtr % 2) * 2:(qtr % 2) * 2 + 2, :],
                    )
                nc.sync.dma_start(w_sb["k"][:], wT3["k"][:])
                nc.scalar.dma_start(w_sb["v"][:], wT3["v"][:])
                for jt in range(1, NJ):
                    for half in range(2):
                        eng = nc.sync if half == 0 else nc.scalar
                        eng.dma_start(
                            xT_sb[:, half * 4:(half + 1) * 4, jt * TQ:(jt + 1) * TQ],
                            xTt[jt, half],
                        )

            QT_sb = const.tile([P, T], BF16, tag="QT", name="QT_sb")
            KT_sb = const.tile([P, T], BF16, tag="KT", name="KT_sb")
            v_sb = const.tile([P, NK, H], BF16, tag="V", name="v_sb")
            ones_sb = const.tile([P, P], BF16, tag="ones", name="ones_sb")
            nc.vector.memset(ones_sb[:], 1.0)

            # ---------- interleaved per tq-block: Q_j, K_j, V_(4j..4j+3), ATT_j ----------
            for j in range(NJ):
                # QT / KT for this block
                for name, dst in (("q", QT_sb), ("k", KT_sb)):
                    ps = qkv_psum.tile([P, TQ], F32, tag="qkvps", name="qkvps")
                    for c in range(NC_CHUNK):
                        nc.tensor.matmul(
                            ps[:],
                            lhsT=w_sb[name][:, c, :],
                            rhs=xT_sb[:, c, j * TQ:(j + 1) * TQ],
                            start=(c == 0),
                            stop=(c == NC_CHUNK - 1),
                        )
                    nc.vector.tensor_copy(dst[:, j * TQ:(j + 1) * TQ], ps[:])

                # V chunks 4j .. 4j+3
                for m in range(4 * j, 4 * j + 4):
                    ps = qkv_psum.tile([P, TQ], F32, tag="qkvps", name="qkvps")
                    for c in range(NC_CHUNK):
                        nc.tensor.matmul(
                            ps[:, :H],
                            lhsT=xT_sb[:, c, m * P:(m + 1) * P],
                            rhs=w_sb["v"][:, c, :],
                            start=(c == 0),
                            stop=(c == NC_CHUNK - 1),
                        )
                    nc.vector.tensor_copy(v_sb[:, m, :], ps[:, :H])

                # attention for tq block j (causal: tk chunks 0 .. 4j+3).
                # st pairs are emitted ahead; ot/rs matmuls for pair p are
                # emitted after st pair p+2, so the PE never waits on the
                # ACT(exp) -> DVE(mul) chase.
                n_i = 4 * j + 4
                n2 = n_i // 2
                ot = ot_psum_pool.tile([P, TQ], F32, tag="ot", name="ot")
                rs = rs_psum_pool.tile([P, TQ], F32, tag="rs", name="rs")
                pts = []

                def emit_otrs(p):
                    for k in range(2):
                        i = 2 * p + k
                        nc.tensor.matmul(
                            ot[:],
                            lhsT=v_sb[:, i, :],
                            rhs=pts[p][:, k, :],
                            start=(i == 0),
                            stop=(i == n_i - 1),
                            skip_group_check=True,
                        )
                    for k in range(2):
                        i = 2 * p + k
                        nc.tensor.matmul(
                            rs[:],
                            lhsT=ones_sb[:],
                            rhs=pts[p][:, k, :],
                            start=(i == 0),
                            stop=(i == n_i - 1),
                            skip_group_check=True,
                        )

                ebq = [None] * (n_i // 4)
                for p in range(n2):
                    if p % 2 == 0:
                        q4 = p // 2
                        ebq[q4] = eb_pool.tile([P, 4, TQ], BF16, tag="eb", name="eb")
                        eb_eng = nc.sync if (j + q4) % 2 == 0 else nc.scalar
                        eb_eng.dma_start(
                            ebq[q4][:],
                            ebt[j, q4].rearrange("p (four q) -> p four q", four=4),
                        )
                    st2 = st_psum_pool.tile([P, 2, TQ], F32, tag="st", name="st2")
                    for k in range(2):
                        i = 2 * p + k
                        nc.tensor.matmul(
                            st2[:, k, :],
                            lhsT=KT_sb[:, i * P:(i + 1) * P],
                            rhs=QT_sb[:, j * TQ:(j + 1) * TQ],
                            start=True,
                            stop=True,
                        )
                    pt = pt_pool.tile([P, 2, TQ], BF16, tag="pt", name="pt")
                    nc.scalar.activation(
                        pt[:], st2[:], mybir.ActivationFunctionType.Exp, scale=SCALE
                    )
                    nc.vector.tensor_mul(
                        pt[:], pt[:], ebq[p // 2][:, (p % 2) * 2:(p % 2) * 2 + 2, :]
                    )
                    pts.append(pt)
                    if p >= 2:
                        emit_otrs(p - 2)
                for p in range(max(0, n2 - 2), n2):
                    emit_otrs(p)

                # 1/rowsum via exp(-ln(x)) on the Scalar engine (ACT tables are
                # accurate to ~1e-6 here; tolerance is 2e-2)
                lnr = out_pool.tile([P, TQ], F32, tag="lnr", name="lnr")
                nc.scalar.activation(lnr[:], rs[:], mybir.ActivationFunctionType.Ln)
                recip = out_pool.tile([P, TQ], F32, tag="recip", name="recip")
                nc.scalar.activation(
                    recip[:], lnr[:], mybir.ActivationFunctionType.Exp, scale=-1.0
                )
                otn = out_pool.tile([P, TQ], F32, tag="otn", name="otn")
                nc.vector.tensor_mul(otn[:], ot[:], recip[:])
                nc.scalar.dma_start(outt[j], otn[:])

    _split_excess_waits(nc)
    return nc


def _get_nc():
    global _nc_cache
    if _nc_cache is None:
        _nc_cache = _build_nc()
    return _nc_cache


def kernel(x, Wq, Wk, Wv, resonance_bias, allowed):
    x = np.asarray(x, dtype=np.float32)
    Wq = np.asarray(Wq, dtype=np.float32)
    Wk = np.asarray(Wk, dtype=np.float32)
    Wv = np.asarray(Wv, dtype=np.float32)
    resonance_bias = np.asarray(resonance_bias, dtype=np.float32)
    allowed = np.asarray(allowed)

    bf16 = ml_dtypes.bfloat16
    eb = np.exp(resonance_bias) * allowed  # exp(log1p(r))*mask = (1+r)*mask, exact
    ebT = eb.T.astype(bf16)                              # [tk, tq]
    # quad-tiled: [j, i4, p, 4*TQ]
    ebt = np.ascontiguousarray(
        ebT.reshape(NK // 4, 4, P, NJ, TQ).transpose(3, 0, 2, 1, 4).reshape(
            NJ, NK // 4, P, 4 * TQ
        )
    )
    wvT = np.ascontiguousarray(Wv.T).astype(bf16)

    wqT = np.ascontiguousarray(Wq.T).astype(bf16)
    wkT = np.ascontiguousarray(Wk.T).astype(bf16)
    in_maps = []
    for b in range(NCORES):
        xTt_b = np.ascontiguousarray(
            x[b].T.astype(bf16)
            .reshape(2, NC_CHUNK // 2, P, NJ, TQ)
            .transpose(3, 0, 2, 1, 4)
        )
        in_maps.append(
            {"xTt": xTt_b, "wqT": wqT, "wkT": wkT, "wvT": wvT, "ebt": ebt}
        )

    nc = _get_nc()
    from concourse import bass2jax

    results = bass2jax.run_bass_via_pjrt(nc, in_maps, n_cores=NCORES)

    out = np.empty((B, T, H), dtype=np.float32)
    for b in range(NCORES):
        outt = results[b]["outt"]                         # [NJ, H, TQ]
        out[b] = outt.transpose(0, 2, 1).reshape(T, H)
    return out


# revision 33
# speedup vs baseline: 1.0114x; 1.0114x over previous
"""Trainium2 Bass kernel for nn_ExactSpectralHead (sparse resonance attention).

Reference computation (per batch element b):
    q = x @ Wq.T; k = x @ Wk.T; v = x @ Wv.T          # [T, H]
    s = (q @ k.T) * C**-0.5 + resonance_bias          # [T, T]
    s = where(allowed, s, -inf); p = softmax(s, -1)
    out = p @ v                                        # [T, H]

Strategy (8 NeuronCores, data-parallel over batch B=8, one b per core):
  - Host folds bias+mask into EB = exp(bias) * allowed (exact: exp(log1p(r)) = 1+r),
    so p_raw = exp(s_qk * scale) * EB with no -inf handling and exact zeros.
    Scores are bounded (|s|<~5), so no max-subtraction is needed; normalization is
    deferred to after the PV matmul via a row-sum.
  - Everything is computed in a transposed layout so that every matmul contracts
    over the partition dim with zero on-device transposes:
      xT [C, T] (host-transposed), QT/KT = W.T^T @ xT -> [H, T],
      ST[tk, tq] = KT.T @ QT, PT = exp(ST*scale) * EBT,
      OT[h, tq] += V[tk,:].T @ PT[tk, tq]   (V in natural [T, H] layout),
      rowsum[tq] = ones.T @ sum_i PT_i, out = (OT / rowsum).T (transpose on host).
  - bf16 matmul inputs (1 cycle/row on the PE), fp32 PSUM accumulation.
  - Causal block skipping: tiles with tk_chunk > tq_block are never touched.
"""

import sys

sys.path.insert(0, "/opt/trn_rl_repo")

import numpy as np
import ml_dtypes

import concourse.bass as bass
import concourse.tile as tile
import concourse.mybir as mybir

# ----------------------------------------------------------------------------
# Workaround for walrus codegen "Too many sync wait commands" on the
# TileContext tail Drain: split the global-clock sem waits across multiple SP
# NOP instructions instead of attaching them all to the single Drain.
from concourse.vector_clock import ScopedClock, VectorClock


def _split_drain_and_barrier(self, tick_clock, wait_clock):
    """Cheap kernel tail: per-proc sem waits split across SP NOPs (walrus
    one-wait-per-instruction limit), then a regular-semaphore all-engine
    completion barrier (the stock EVSEM butterfly costs ~1.5-4us per hop),
    then GpSimd clears the tile semaphores. The next NEFF execution cannot
    start until every engine stream (including the clear) retires, so no
    trailing barrier is needed."""
    import concourse.mybir as _mybir

    nc = self.nc
    gc = tick_clock.global_clock
    n = len(gc)
    for p in range(n):
        t = gc[p]
        if t > 0:
            nop = nc.sync.nop(nofuse=True, hint=f"drain_wait_{p}")
            vc = VectorClock([t if i == p else 0 for i in range(n)])
            wait_clock.add_sem_waits(nop.ins, ScopedClock({None: vc}))

    tail_sem = nc.alloc_semaphore("tile_tail_sem")
    n_signals = 0
    for etype, eng in nc.engines.items():
        if etype == _mybir.EngineType.Pool:
            continue
        eng.drain(fusable=False)
        eng.sem_inc(tail_sem, 1)
        n_signals += 1
    nc.gpsimd.wait_ge(tail_sem, n_signals)
    assert self.sems is not None
    popped = nc._tile_sem_poison_stack.pop()
    assert popped is self._sem_poison
    nc.clear_and_free_semaphores(list(self.sems.allocated().values()))
    nc.gpsimd.sem_clear(range(tail_sem.num, tail_sem.num + 1))


tile.TileContext._drain_and_barrier = _split_drain_and_barrier
# ----------------------------------------------------------------------------

def _split_excess_waits(nc, max_waits=1):
    """Walrus codegen in this toolchain supports only one sem-wait per
    instruction; hoist excess waits onto preceding same-engine NOPs."""
    for f in nc.m.functions:
        for bb in f.blocks:
            new = []
            changed = False
            for inst in bb.instructions:
                if isinstance(inst, mybir.InstEventSemaphore):
                    # EventSemaphore ops measure ~3-5us on HW; their barrier
                    # semantics live entirely in sync_info (regular sems), so
                    # NoOps with the same sync_info are equivalent and fast.
                    # Waits and updates go on separate NoOps (wait first) to
                    # satisfy the no_semaphore_value_conflict ISA check.
                    si = inst.sync_info
                    changed = True
                    w = list(si.on_wait) if si else []
                    u = list(si.on_update) if si else []
                    if w:
                        new.append(
                            mybir.InstNoOp(
                                name=f"{inst.name}-wait",
                                engine=inst.engine,
                                bass_nofuse=True,
                                sync_info=mybir.SyncInfo(on_wait=w, on_update=[]),
                            )
                        )
                    new.append(
                        mybir.InstNoOp(
                            name=inst.name,
                            engine=inst.engine,
                            bass_nofuse=True,
                            sync_info=mybir.SyncInfo(on_wait=[], on_update=u),
                        )
                    )
                    continue
                si = inst.sync_info
                waits = list(si.on_wait) if si is not None else []
                if len(waits) > max_waits:
                    changed = True
                    excess, keep = waits[:-max_waits], waits[-max_waits:]
                    for k, w in enumerate(excess):
                        new.append(
                            mybir.InstNoOp(
                                name=f"{inst.name}-w{k}",
                                engine=inst.engine,
                                bass_nofuse=True,
                                sync_info=mybir.SyncInfo(on_wait=[w], on_update=[]),
                            )
                        )
                    inst.sync_info = mybir.SyncInfo(
                        on_wait=keep, on_update=list(si.on_update)
                    )
                new.append(inst)
            if changed:
                bb.instructions = new


B, T, C, H = 8, 2048, 1024, 128
NCORES = 8
SCALE = float(C) ** -0.5
P = 128
TQ = 512                 # tq block width (matmul moving dim)
NJ = T // TQ             # 4 tq blocks
NC_CHUNK = C // P        # 8 contraction chunks over channels
NK = T // P              # 16 tk chunks
BF16 = mybir.dt.bfloat16
FP8 = mybir.dt.float8e4
F32 = mybir.dt.float32
F32R = mybir.dt.float32r

_nc_cache = None


def _build_nc():
    nc = bass.Bass()
    # xT tiled: [jt, c, 128, TQ] so each (c, jt) load is one contiguous 128KB
    # per (jt, half): contiguous 512KB, partition-major [p, 4c, q]
    xTt = nc.declare_dram_parameter("xTt", [NJ, 2, P, NC_CHUNK // 2, TQ], BF16, isOutput=False)
    wqT = nc.declare_dram_parameter("wqT", [C, H], BF16, isOutput=False)
    wkT = nc.declare_dram_parameter("wkT", [C, H], BF16, isOutput=False)
    wvT = nc.declare_dram_parameter("wvT", [C, H], BF16, isOutput=False)
    # ebT quad-tiled: [j, i4, 128, 4*TQ] with ebt[j,i4,p,k*TQ+q] = EB.T[128*(4*i4+k)+p, j*TQ+q]
    ebt = nc.declare_dram_parameter("ebt", [NJ, NK // 4, P, 4 * TQ], BF16, isOutput=False)
    # output tiled: [j, 128, TQ] (host reassembles)
    outt = nc.declare_dram_parameter("outt", [NJ, H, TQ], F32, isOutput=True)

    wT3 = {
        "q": wqT.rearrange("(o p) h -> p o h", p=P),
        "k": wkT.rearrange("(o p) h -> p o h", p=P),
        "v": wvT.rearrange("(o p) h -> p o h", p=P),
    }

    with tile.TileContext(nc) as tc:
        with (
            tc.tile_pool(name="const", bufs=1) as const,
            tc.tile_pool(name="qkv_psum", bufs=2, space="PSUM") as qkv_psum,
            tc.tile_pool(name="st_psum", bufs=2, space="PSUM") as st_psum_pool,
            tc.tile_pool(name="ot_psum", bufs=1, space="PSUM") as ot_psum_pool,
            tc.tile_pool(name="rs_psum", bufs=1, space="PSUM") as rs_psum_pool,
            tc.tile_pool(name="pt", bufs=NK // 2 + 2) as pt_pool,
            tc.tile_pool(name="eb", bufs=4) as eb_pool,
            tc.tile_pool(name="outs", bufs=2) as out_pool,
        ):
            # ---------- persistent SBUF tensors ----------
            w_sb = {}
            for w in ("q", "k", "v"):
                w_sb[w] = const.tile([P, NC_CHUNK, H], BF16, tag=f"w_{w}", name=f"w_{w}_sb")
            with tc.high_priority():
                nc.gpsimd.dma_start(w_sb["q"][:], wT3["q"][:])
                nc.gpsimd.dma_start(w_sb["k"][:], wT3["k"][:])

            # xT loaded per (tq-block, c-chunk) in consumption order
            xT_sb = const.tile([P, NC_CHUNK, T], BF16, tag="xT", name="xT_sb")
            with tc.high_priority():
                for jt in range(NJ):
                    for half in range(2):
                        eng = nc.sync if half == 0 else nc.scalar
                        eng.dma_start(
                            xT_sb[:, half * 4:(half + 1) * 4, jt * TQ:(jt + 1) * TQ],
                            xTt[jt, half],
                        )
                    if jt == 0:
                        nc.gpsimd.dma_start(w_sb["v"][:], wT3["v"][:])

            QT_sb = const.tile([P, T], BF16, tag="QT", name="QT_sb")
            KT_sb = const.tile([P, T], BF16, tag="KT", name="KT_sb")
            v_sb = const.tile([P, NK, H], BF16, tag="V", name="v_sb")
            ones_sb = const.tile([P, P], BF16, tag="ones", name="ones_sb")
            nc.vector.memset(ones_sb[:], 1.0)

            # ---------- interleaved per tq-block: Q_j, K_j, V_(4j..4j+3), ATT_j ----------
            for j in range(NJ):
                # QT / KT for this block
                for name, dst in (("q", QT_sb), ("k", KT_sb)):
                    ps = qkv_psum.tile([P, TQ], F32, tag="qkvps", name="qkvps")
                    for c in range(NC_CHUNK):
                        nc.tensor.matmul(
                            ps[:],
                            lhsT=w_sb[name][:, c, :],
                            rhs=xT_sb[:, c, j * TQ:(j + 1) * TQ],
                            start=(c == 0),
                            stop=(c == NC_CHUNK - 1),
                        )
                    nc.vector.tensor_copy(dst[:, j * TQ:(j + 1) * TQ], ps[:])

                # V chunks 4j .. 4j+3
                for m in range(4 * j, 4 * j + 4):
                    ps = qkv_psum.tile([P, TQ], F32, tag="qkvps", name="qkvps")
                    for c in range(NC_CHUNK):
                        nc.tensor.matmul(
                            ps[:, :H],
                            lhsT=xT_sb[:, c, m * P:(m + 1) * P],
                            rhs=w_sb["v"][:, c, :],
                            start=(c == 0),
                            stop=(c == NC_CHUNK - 1),
                        )
                    nc.vector.tensor_copy(v_sb[:, m, :], ps[:, :H])

                # attention for tq block j (causal: tk chunks 0 .. 4j+3).
                # st pairs are emitted ahead; ot/rs matmuls for pair p are
                # emitted after st pair p+2, so the PE never waits on the
                # ACT(exp) -> DVE(mul) chase.
                n_i = 4 * j + 4
                n2 = n_i // 2
                ot = ot_psum_pool.tile([P, TQ], F32, tag="ot", name="ot")
                rs = rs_psum_pool.tile([P, TQ], F32, tag="rs", name="rs")
                pts = []

                def emit_otrs(p):
                    for k in range(2):
                        i = 2 * p + k
                        nc.tensor.matmul(
                            ot[:],
                            lhsT=v_sb[:, i, :],
                            rhs=pts[p][:, k, :],
                            start=(i == 0),
                            stop=(i == n_i - 1),
                            skip_group_check=True,
                        )
                    for k in range(2):
                        i = 2 * p + k
                        nc.tensor.matmul(
                            rs[:],
                            lhsT=ones_sb[:],
                            rhs=pts[p][:, k, :],
                            start=(i == 0),
                            stop=(i == n_i - 1),
                            skip_group_check=True,
                        )

                ebq = [None] * (n_i // 4)
                for p in range(n2):
                    if p % 2 == 0:
                        q4 = p // 2
                        ebq[q4] = eb_pool.tile([P, 4, TQ], BF16, tag="eb", name="eb")
                        eb_eng = nc.sync if (j + q4) % 2 == 0 else nc.scalar
                        eb_eng.dma_start(
                            ebq[q4][:],
                            ebt[j, q4].rearrange("p (four q) -> p four q", four=4),
                        )
                    st2 = st_psum_pool.tile([P, 2, TQ], F32, tag="st", name="st2")
                    for k in range(2):
                        i = 2 * p + k
                        nc.tensor.matmul(
                            st2[:, k, :],
                            lhsT=KT_sb[:, i * P:(i + 1) * P],
                            rhs=QT_sb[:, j * TQ:(j + 1) * TQ],
                            start=True,
                            stop=True,
                        )
                    pt = pt_pool.tile([P, 2, TQ], BF16, tag="pt", name="pt")
                    nc.scalar.activation(
                        pt[:], st2[:], mybir.ActivationFunctionType.Exp, scale=SCALE
                    )
                    nc.vector.tensor_mul(
                        pt[:], pt[:], ebq[p // 2][:, (p % 2) * 2:(p % 2) * 2 + 2, :]
                    )
                    pts.append(pt)
                    if p >= 2:
                        emit_otrs(p - 2)
                for p in range(max(0, n2 - 2), n2):
                    emit_otrs(p)

                # 1/rowsum via exp(-ln(x)) on the Scalar engine (ACT tables are
                # accurate to ~1e-6 here; tolerance is 2e-2)
                lnr = out_pool.tile([P, TQ], F32, tag="lnr", name="lnr")
                nc.scalar.activation(lnr[:], rs[:], mybir.ActivationFunctionType.Ln)
                recip = out_pool.tile([P, TQ], F32, tag="recip", name="recip")
                nc.scalar.activation(
                    recip[:], lnr[:], mybir.ActivationFunctionType.Exp, scale=-1.0
                )
                otn = out_pool.tile([P, TQ], F32, tag="otn", name="otn")
                nc.vector.tensor_mul(otn[:], ot[:], recip[:])
                nc.scalar.dma_start(outt[j], otn[:])

    _split_excess_waits(nc)
    return nc


def _get_nc():
    global _nc_cache
    if _nc_cache is None:
        _nc_cache = _build_nc()
    return _nc_cache


def kernel(x, Wq, Wk, Wv, resonance_bias, allowed):
    x = np.asarray(x, dtype=np.float32)
    Wq = np.asarray(Wq, dtype=np.float32)
    Wk = np.asarray(Wk, dtype=np.float32)
    Wv = np.asarray(Wv, dtype=np.float32)
    resonance_bias = np.asarray(resonance_bias, dtype=np.float32)
    allowed = np.asarray(allowed)

    bf16 = ml_dtypes.bfloat16
    eb = np.exp(resonance_bias) * allowed  # exp(log1p(r))*mask = (1+r)*mask, exact
    ebT = eb.T.astype(bf16)                              # [tk, tq]
    # quad-tiled: [j, i4, p, 4*TQ]
    ebt = np.ascontiguousarray(
        ebT.reshape(NK // 4, 4, P, NJ, TQ).transpose(3, 0, 2, 1, 4).reshape(
            NJ, NK // 4, P, 4 * TQ
        )
    )
    wvT = np.ascontiguousarray(Wv.T).astype(bf16)

    wqT = np.ascontiguousarray(Wq.T).astype(bf16)
    wkT = np.ascontiguousarray(Wk.T).astype(bf16)
    in_maps = []
    for b in range(NCORES):
        xTt_b = np.ascontiguousarray(
            x[b].T.astype(bf16)
            .reshape(2, NC_CHUNK // 2, P, NJ, TQ)
            .transpose(3, 0, 2, 1, 4)
        )
        in_maps.append(
            {"xTt": xTt_b, "wqT": wqT, "wkT": wkT, "wvT": wvT, "ebt": ebt}
        )

    nc = _get_nc()
    from concourse import bass2jax

    results = bass2jax.run_bass_via_pjrt(nc, in_maps, n_cores=NCORES)

    out = np.empty((B, T, H), dtype=np.float32)
    for b in range(NCORES):
        outt = results[b]["outt"]                         # [NJ, H, TQ]
        out[b] = outt.transpose(0, 2, 1).reshape(T, H)
    return out
